# revision 9
# baseline (speedup 1.0000x reference)
"""Trainium2 Bass kernel for int4-quantized einsum:
    out[b,t,f] = sum_d x[b,t,d] * (int4_kernel[d,f] / scale[f])

Strategy (column-parallel over 8 NeuronCores):
  - shard kernel & scale along f (2048 per core), replicate x
  - per core: out = (x @ k_int) * (1/scale), computed as a PE matmul with
    x^T (bf16) as the stationary operand and k (fp8e4, exact for int4) as
    the moving operand, fp32 PSUM accumulation, per-channel scaling on DVE
  - everything is SBUF-resident (x^T 64KB/p + w 64KB/p), so all HBM bytes
    move exactly once and DMA is hidden behind PE work
"""

import numpy as np
import ml_dtypes

P = 128
B, T, D, F = 1, 1024, 4096, 16384
NCORES = 8
FS = F // NCORES          # 2048 output features per core
NCH = D // P              # 32 contraction chunks
NTT = T // P              # 8 token tiles
NQ = FS // 512            # 4 psum-bank tiles per token tile
DMA_GROUPS = 32           # input DMAs split for pipelined start
WARMUP_MMS = 16           # dummy matmuls to ramp the PE clock during DMA fill
OUT_SPLIT = 4             # out-DMAs per token tile (1, 2 or 4)
FINE_CHUNKS = 0           # leading input chunks DMA'd singly for a fast start
PAIR_FIRST = 1            # process tt0+tt1 as one 8-bank pair (hides DMA fill)
LAST_QOUTER = 1           # last token tile: q-outer/c-inner order (shorter tail)

_cached = {}
last_results = None       # BassKernelResults of the most recent run (for test.py)


def _build_bass():
    import concourse.bacc as bacc
    import concourse.mybir as mybir
    import concourse.tile as tile

    nc = bacc.Bacc(None, target_bir_lowering=False, debug=False, enable_asserts=False)

    xt_d = nc.dram_tensor("xt", [P, NCH * T], mybir.dt.bfloat16, kind="ExternalInput")
    w_d = nc.dram_tensor("w", [P, NCH * FS], mybir.dt.float8e4, kind="ExternalInput")
    rs_d = nc.dram_tensor("rs", [P, FS], mybir.dt.float32, kind="ExternalInput")
    out_d = nc.dram_tensor("out", [P, NTT * FS], mybir.dt.float32, kind="ExternalOutput")

    with tile.TileContext(nc) as tc:
        with (
            tc.tile_pool(name="big", bufs=1) as big,
            tc.tile_pool(name="outp", bufs=3) as outp,
            tc.tile_pool(name="psum", bufs=8, space="PSUM") as psump,
        ):
            xt_sb = big.tile([P, NCH * T], mybir.dt.bfloat16)
            w_sb = big.tile([P, NCH * FS], mybir.dt.float8e4)
            rs_sb = big.tile([P, FS], mybir.dt.float32)

            # PE warmup: dummy matmuls with no DMA deps run while the first
            # input chunks stream in, flipping the HAM clock gate to 8/8 so
            # real matmuls start at 2.4 GHz.
            if WARMUP_MMS:
                wu = big.tile([P, 512], mybir.dt.bfloat16)
                nc.any.memset(wu[:, :], 0.0)
                wups = psump.tile([P, 512], mybir.dt.float32, tag="ps", name="ps")
                for i in range(WARMUP_MMS):
                    nc.tensor.matmul(
                        wups[:, 0:256], wu[:, 0:P], wu[:, 0:256],
                        start=(i == 0), stop=(i == WARMUP_MMS - 1),
                    )

            # interleave x/w input DMAs so chunk c of both arrives in order;
            # the first FINE_CHUNKS chunks go as single-chunk DMAs so the
            # first matmul starts as early as possible
            bounds = list(range(FINE_CHUNKS + 1)) if FINE_CHUNKS else [0]
            rest = NCH - bounds[-1]
            per = rest // DMA_GROUPS
            acc = bounds[-1]
            for g in range(DMA_GROUPS):
                acc += per + (1 if g < rest % DMA_GROUPS else 0)
                bounds.append(acc)
            for lo, hi in zip(bounds[:-1], bounds[1:]):
                nc.sync.dma_start(
                    xt_sb[:, lo * T : hi * T], xt_d[:, lo * T : hi * T]
                )
                nc.sync.dma_start(
                    w_sb[:, lo * FS : hi * FS], w_d[:, lo * FS : hi * FS]
                )
                if hi >= 8 and lo < 8:
                    nc.sync.dma_start(rs_sb[:, :], rs_d[:, :])

            def mm(ps, tt, c, q):
                nc.tensor.matmul(
                    ps[:, :],
                    xt_sb[:, c * T + tt * P : c * T + (tt + 1) * P],
                    w_sb[:, c * FS + q * 512 : c * FS + (q + 1) * 512],
                    start=(c == 0),
                    stop=(c == NCH - 1),
                )

            def drain(psums, tt):
                ot = outp.tile([P, FS], mybir.dt.float32, tag="ot", name="ot")
                qper = NQ // OUT_SPLIT
                for g in range(OUT_SPLIT):
                    for q in range(g * qper, (g + 1) * qper):
                        nc.vector.tensor_mul(
                            out=ot[:, q * 512 : (q + 1) * 512],
                            in0=psums[q][:, :],
                            in1=rs_sb[:, q * 512 : (q + 1) * 512],
                        )
                    lo = tt * FS + g * qper * 512
                    hi = tt * FS + (g + 1) * qper * 512
                    nc.sync.dma_start(
                        out_d[:, lo:hi], ot[:, g * qper * 512 : (g + 1) * qper * 512]
                    )

            groups = [[0, 1]] + [[t] for t in range(2, NTT)] if PAIR_FIRST else [
                [t] for t in range(NTT)
            ]
            for gi, group in enumerate(groups):
                psums = {
                    tt: [
                        psump.tile([P, 512], mybir.dt.float32, tag="ps", name="ps")
                        for _ in range(NQ)
                    ]
                    for tt in group
                }
                if LAST_QOUTER and gi == len(groups) - 1:
                    for tt in group:
                        for q in range(NQ):
                            for c in range(NCH):
                                mm(psums[tt][q], tt, c, q)
                else:
                    for c in range(NCH):
                        for tt in group:
                            for q in range(NQ):
                                mm(psums[tt][q], tt, c, q)
                for tt in group:
                    drain(psums[tt], tt)
    nc.compile()
    return nc


def _get_nc():
    if "nc" not in _cached:
        _cached["nc"] = _build_bass()
    return _cached["nc"]


def _prep_in_maps(x, k, s):
    """Host-side layout prep. x [T,D] f32, k [D,F] f32(int vals), s [1,F] f32."""
    bf16 = ml_dtypes.bfloat16
    fp8 = ml_dtypes.float8_e4m3

    # x^T, chunk-major: xt[p, c*T + t] = x[t, c*P + p]
    xt = np.ascontiguousarray(
        x.T.reshape(NCH, P, T).transpose(1, 0, 2).reshape(P, NCH * T)
    ).astype(bf16)
    recip = (1.0 / s.astype(np.float64)).astype(np.float32)  # [1, F]

    in_maps = []
    for core in range(NCORES):
        ks = k[:, core * FS : (core + 1) * FS]  # [D, FS]
        w8 = np.ascontiguousarray(
            ks.reshape(NCH, P, FS).transpose(1, 0, 2).reshape(P, NCH * FS)
        ).astype(fp8)
        rs = np.ascontiguousarray(
            np.broadcast_to(recip[:, core * FS : (core + 1) * FS], (P, FS))
        ).astype(np.float32)
        in_maps.append({"xt": xt, "w": w8, "rs": rs})
    return in_maps


def _assemble(results):
    out = np.empty((T, F), np.float32)
    for core in range(NCORES):
        o = np.asarray(results[core]["out"])  # [P, NTT*FS]
        out[:, core * FS : (core + 1) * FS] = (
            o.reshape(P, NTT, FS).transpose(1, 0, 2).reshape(T, FS)
        )
    return out.reshape(B, T, F)


def kernel(inputs, kernel, scale):
    global last_results
    from concourse.bass_utils import run_bass_kernel_spmd

    x = np.asarray(inputs).reshape(T, D).astype(np.float32)
    k = np.asarray(kernel).astype(np.int8).astype(np.float32)  # [D, F]
    s = np.asarray(scale).reshape(1, F).astype(np.float32)

    in_maps = _prep_in_maps(x, k, s)
    nc = _get_nc()
    res = run_bass_kernel_spmd(nc, in_maps, core_ids=list(range(NCORES)))
    last_results = res
    return _assemble(res.results)
